# revision 28
# baseline (speedup 1.0000x reference)
"""Bahdanau attention kernel for Trainium2 (Bass/Tile), 8-core data-parallel.

Problem shapes: B=32, Tx=1024, enc_hid=dec_hid=attn=1024, fp32 in/out.

Math (per example b):
  dec_proj = W_dec @ dec_hidden[b]                 [attn]
  energy^T[a, t] = tanh(sum_e W_enc[a,e] enc[b,t,e] + dec_proj[a] + W_b[a])
  scores[t] = sum_a v[a] energy^T[a, t]
  alpha = softmax(mask(scores))
  context[e] = sum_t alpha[t] enc[b,t,e]

Sharding: batch B split 4 examples per core across 8 cores; weights replicated.

The PE's only steady-state work is the energy matmuls (bf16, N=512 moving,
216ns warm roofline).  Everything else is moved off the PE:

* scores: v-weighted partition-chunk accumulation runs on the DVE via
  scalar_tensor_tensor (acc = en*v_chunk + acc, one op per (ao, nt)); the
  final cross-partition sum is ONE f32r ones-matmul per 512-wide half.  The
  mask penalty rides in the ao=0 accumulation's in1 (a tile that is zero
  except row 0 = -100*(1-mask)), so the softmax exp reads the score PSUM
  directly.  This replaces 64+8 PE matmuls per core with 8.
* context (b=0..2): fused tensor_tensor_reduce against the resident encT
  tiles (alpha broadcast by a K=1 matmul pair), evacuated via one PE
  transpose.  b=3 (the serial tail) splits e-halves across PE (transposed-exp
  matmuls on natural-layout enc3) and DVE (tensor_tensor_reduce on encT3),
  with 1/sum folded into both evacuations.

Scores are bounded (|s| <= sum|v| ~ 26 since |tanh| <= 1) so exp needs no
max shift -- softmax is shift-invariant; masked lanes get exp(s-100) ~ 1e-33
which is negligible vs the 2e-2 gate.

Ramp: the first example's tiles (w_encT[eo] + encT0[eo], full 2KB rows) are
interleaved round-robin over the three DMA-capable engine queues
(sync/gpsimd/scalar) so the PE is densely fed from ~9us and the HAM clock-gate releases
~3.4us later; small tensors and w_decT follow behind the critical pairs.
dec_proj (at=0/at=1 halves) is emitted AFTER the first two energy passes so
the PE never head-of-line blocks on the w_decT stream.
"""

from contextlib import ExitStack

import numpy as np

import concourse.bass as bass
import concourse.tile as tile
from concourse import bacc, mybir
from concourse.masks import make_identity

F32 = mybir.dt.float32
F32R = mybir.dt.float32r
BF16 = mybir.dt.bfloat16
AF = mybir.ActivationFunctionType
ALU = mybir.AluOpType

P = 128
N_CORES = 8
B_LOC = 4            # examples per core
TX = 1024
E = 1024             # enc_hid
A = 1024             # attn
D = 1024             # dec_hid
EO = E // P          # e-chunks
AO = A // P          # a-chunks
DO = D // P          # d-chunks
NT = TX // 512       # t-tiles for energy free dim
TO = TX // P         # t-chunks on partitions (tail context)
MASK_PEN = 100.0     # additive penalty on masked scores (|s| <= ~26)


def build_nc():
    nc = bacc.Bacc(
        "TRN2", target_bir_lowering=False, debug=False, num_devices=N_CORES
    )
    encT = nc.dram_tensor("encT", [B_LOC, E, TX], BF16, kind="ExternalInput").ap()
    enc3 = nc.dram_tensor("enc3", [TX, E], BF16, kind="ExternalInput").ap()
    w_encT = nc.dram_tensor("w_encT", [E, A], BF16, kind="ExternalInput").ap()
    w_decT = nc.dram_tensor("w_decT", [D, A], BF16, kind="ExternalInput").ap()
    dec_hT = nc.dram_tensor("dec_hT", [D, B_LOC], BF16, kind="ExternalInput").ap()
    v8 = nc.dram_tensor("v8", [P, AO], F32, kind="ExternalInput").ap()
    wb8 = nc.dram_tensor("wb8", [P, AO], F32, kind="ExternalInput").ap()
    mb = nc.dram_tensor("mb", [1, B_LOC * TX], F32, kind="ExternalInput").ap()
    ctx_out = nc.dram_tensor("context", [B_LOC, E], F32, kind="ExternalOutput").ap()
    alpha_out = nc.dram_tensor("alpha", [B_LOC, TX], F32, kind="ExternalOutput").ap()

    with tile.TileContext(nc) as tc, ExitStack() as ctx:
        const = ctx.enter_context(tc.tile_pool(name="const", bufs=1))
        big = ctx.enter_context(tc.tile_pool(name="big", bufs=6))
        en_pool = ctx.enter_context(tc.tile_pool(name="energy", bufs=6))
        small = ctx.enter_context(tc.tile_pool(name="small", bufs=2))
        rowp = ctx.enter_context(tc.tile_pool(name="rows", bufs=2))
        ep_psum = ctx.enter_context(tc.tile_pool(name="ep_ps", bufs=4, space="PSUM"))
        vec_psum = ctx.enter_context(tc.tile_pool(name="vec_ps", bufs=4, space="PSUM"))

        # ---- tiles (all resident for the whole kernel; nothing slot-gated)
        w_encT_sb = const.tile([P, EO, A], BF16)
        encT_tiles = [
            big.tile([P, EO, TX], BF16, tag="big", name=f"encT_sb{b}")
            for b in range(B_LOC)
        ]
        w_decT_sb = big.tile([P, DO, A], BF16, tag="big", name="w_decT_sb")
        enc3_sb = big.tile([P, TO, E], BF16, tag="big", name="enc3_sb")
        dec_hT_sb = const.tile([P, DO, B_LOC], BF16)
        v_sb = const.tile([P, AO], F32)
        wb_sb = const.tile([P, AO], F32)
        mb_sb = const.tile([1, B_LOC * TX], F32)
        bias_sb = const.tile([P, AO, B_LOC], F32)
        # score accumulators + mask-carrying init tiles (zero except row 0)
        # ping-pong accumulator pairs: no DVE op reads and writes the
        # same address (out alternates between the two F32 tiles; the last
        # chunk lands in the f32r staging tile the ones-matmul reads)
        acc_nt = [
            [const.tile([P, 512], F32, name=f"acc{nt}_{i}") for i in range(2)]
            for nt in range(NT)
        ]
        # bf16 hi/lo split of the final accumulator: the ones-matmul sums
        # hi and lo in two accumulating bf16 matmuls (full f32 precision,
        # no f32r weight loads on the PE)
        acc_hi = [const.tile([P, 512], BF16, name=f"acchi{nt}") for nt in range(NT)]
        acc_lo = [const.tile([P, 512], BF16, name=f"acclo{nt}") for nt in range(NT)]
        vtmp = [
            [const.tile([P, 512], BF16, name=f"vtmp{nt}_{i}") for i in range(2)]
            for nt in range(NT)
        ]
        mbinit = [const.tile([P, 512], F32, name=f"mbi{nt}") for nt in range(NT)]
        ident128 = const.tile([P, P], F32)
        ones_row = const.tile([1, P], BF16)
        ones_col = const.tile([P, 1], BF16)
        ident1 = const.tile([1, 1], BF16)

        # ---- vector-queue constants (before its DMAs)
        nc.vector.memset(mbinit[0][:], 0.0)
        nc.vector.memset(mbinit[1][:], 0.0)
        nc.vector.memset(ones_row[:], 1.0)
        nc.vector.memset(ones_col[:], 1.0)
        nc.vector.memset(ident1[:], 1.0)

        # identity for PE transposes: gpsimd custom ops go first on its
        # queue (before its DMA stream), as in the proven baseline ordering
        make_identity(nc, ident128[:])

        # ---- ramp-critical DMA: (w_encT[eo], encT0[eo]) pairs round-robin
        # over all three DMA-capable engine queues (sync/gpsimd/scalar);
        # encT0 in nt-halves so each eo's first matmuls start while the
        # second half is in flight.
        lanes3 = [nc.sync, nc.gpsimd, nc.scalar]
        def pair(eo, eng):
            eng.dma_start(w_encT_sb[:, eo], w_encT[eo * P : (eo + 1) * P, :])
            eng.dma_start(
                encT_tiles[0][:, eo, 0:512], encT[0, eo * P : (eo + 1) * P, 0:512]
            )
            eng.dma_start(
                encT_tiles[0][:, eo, 512:1024],
                encT[0, eo * P : (eo + 1) * P, 512:1024],
            )
        def wdec(do, eng):
            eng.dma_start(w_decT_sb[:, do], w_decT[do * P : (do + 1) * P, :])
        # sync:   eo0 | dec_hT wb8 | eo3 eo5 | wdec 0,2,4,6
        # gpsimd: eo1 | v8 mb      | eo4 eo6 | wdec 1,3,5,7
        # scalar: [act table] eo2 eo7 -- free well before the tanh chain
        pair(0, nc.sync); pair(1, nc.gpsimd)
        nc.sync.dma_start(
            dec_hT_sb[:], dec_hT.rearrange("(do p) b -> p do b", p=P)
        )
        nc.sync.dma_start(wb_sb[:], wb8[:])
        nc.gpsimd.dma_start(v_sb[:], v8[:])
        nc.gpsimd.dma_start(mb_sb[:], mb[:])
        pair(2, nc.scalar)
        pair(3, nc.sync); pair(4, nc.gpsimd)
        pair(5, nc.sync); pair(6, nc.gpsimd)
        pair(7, nc.scalar)
        for do in range(DO):
            wdec(do, nc.sync if do % 2 == 0 else nc.gpsimd)
        # remaining example tiles + natural-layout enc for the b=3 tail
        for b in range(1, B_LOC):
            for eo in range(EO):
                eng = nc.sync if eo % 2 == 0 else nc.gpsimd
                eng.dma_start(
                    encT_tiles[b][:, eo], encT[b, eo * P : (eo + 1) * P, :]
                )
            if b == 1:
                nc.gpsimd.dma_start(
                    enc3_sb[:], enc3.rearrange("(to p) e -> p to e", p=P)
                )

        # b=0's mask rows must be on the vector queue before the ramp's ao=0
        # accumulation reads the init tiles
        for nt in range(NT):
            nc.vector.tensor_copy(
                mbinit[nt][0:1, :], mb_sb[0:1, nt * 512 : (nt + 1) * 512]
            )

        dp_row = rowp.tile([B_LOC, A], F32, tag="row4k", name="dp_row")
        fill0 = vec_psum.tile([P, 512], F32, tag="vec", bufs=2, name="fill0")

        def dec_proj_pass(at):
            # dec_proj for all 4 examples at once: out [4, 512] per half,
            # accumulated over the 8 d-chunks; then PE transposes per
            # a-chunk land bias[a-part, b] = dec_proj^T + W_b.
            dp_ps = vec_psum.tile([P, 512], F32, tag="vec", bufs=2, name=f"dp_ps{at}")
            for do in range(DO):
                nc.tensor.matmul(
                    dp_ps[:B_LOC, :],
                    lhsT=dec_hT_sb[:, do],
                    rhs=w_decT_sb[:, do, at * 512 : (at + 1) * 512],
                    start=(do == 0),
                    stop=(do == DO - 1),
                )
                if at == 0 and do < 6:
                    # w_decT rows stream in behind the pairs; keep the PE warm
                    nc.tensor.matmul(
                        fill0[:, :],
                        lhsT=w_encT_sb[:, do, 0:128],
                        rhs=encT_tiles[0][:, do, 0:512],
                        start=True,
                        stop=True,
                    )
            nc.vector.tensor_copy(
                dp_row[:, at * 512 : (at + 1) * 512], dp_ps[:B_LOC, :]
            )
            for ao in range(at * 4, at * 4 + 4):
                tp_ps = vec_psum.tile([P, B_LOC], F32, tag="vec", bufs=2, name=f"tp{ao}")
                nc.tensor.transpose(
                    tp_ps[:], dp_row[:, ao * P : (ao + 1) * P], ident128[:B_LOC, :B_LOC]
                )
                nc.vector.tensor_scalar_add(
                    bias_sb[:, ao], tp_ps[:], wb_sb[:, ao : ao + 1]
                )

        # ---- per-example pipeline -----------------------------------------
        # The PE queue is in-order; anything that waits on a non-PE producer
        # is emitted at least one ao-pass after that producer's input was
        # ready: example b's score ones-matmuls are emitted inside example
        # b+1's pass 0, its context broadcast at pass 1, the context-store
        # transpose at pass 5.
        pend_ones = None
        pend_ctx = None
        pend_ctx_tp = None

        def emit_energy(b, ao, eps):
            encT_sb = encT_tiles[b]
            ens = []
            for nt in range(NT):
                for eo in range(EO):
                    nc.tensor.matmul(
                        eps[nt][:],
                        lhsT=w_encT_sb[:, eo, ao * P : (ao + 1) * P],
                        rhs=encT_sb[:, eo, nt * 512 : (nt + 1) * 512],
                        start=(eo == 0),
                        stop=(eo == EO - 1),
                    )
                energy = en_pool.tile(
                    [P, 512], BF16, tag="energy", name=f"en{b}_{ao}_{nt}"
                )
                nc.scalar.activation(
                    energy[:], eps[nt][:], AF.Tanh, bias=bias_sb[:, ao, b : b + 1]
                )
                ens.append(energy)
            return ens

        def emit_acc(ao, ens, last=False, on_dve=False):
            # DVE: acc += v[a-chunk] * en; ao=0 reads the mask-init tile
            for nt in range(NT):
                out = acc_nt[nt][ao % 2][:]
                in1 = mbinit[nt][:] if ao == 0 else acc_nt[nt][(ao - 1) % 2][:]
                vt = vtmp[nt][ao % 2]
                if last or on_dve:
                    # last pass of b=3: keep the scalar queue clear for exp.
                    # b=0's early passes: the DVE is idle (no context work
                    # yet) while the ramp's scalar tanh chain is the pass
                    # gate -- let it catch up.
                    nc.vector.tensor_scalar_mul(
                        vt[:], ens[nt][:], v_sb[:, ao : ao + 1]
                    )
                else:
                    nc.scalar.mul(vt[:], ens[nt][:], v_sb[:, ao : ao + 1])
                nc.vector.tensor_tensor(
                    out=out, in0=vt[:], in1=in1, op=ALU.add
                )

        # ---- b=0 ramp: passes ao=0,1 run eo-OUTER so each arriving
        # (w_encT[eo], encT0[eo]) pair feeds 4 matmuls and the PE banks two
        # passes of work while the DMA stream lands.  dec_proj at=0 slots in
        # before the final e-chunk so bias[0..3] is ready the moment the
        # accumulation groups close (its w_decT half lands behind the pairs).
        eps01 = [
            [
                ep_psum.tile([P, 512], F32, tag="ep", name=f"ep0_{ao}_{nt}")
                for nt in range(NT)
            ]
            for ao in range(2)
        ]
        for eo in range(EO):
            if eo == EO - 1:
                dec_proj_pass(0)
            for nt in range(NT):
                for ao in range(2):
                    nc.tensor.matmul(
                        eps01[ao][nt][:],
                        lhsT=w_encT_sb[:, eo, ao * P : (ao + 1) * P],
                        rhs=encT_tiles[0][:, eo, nt * 512 : (nt + 1) * 512],
                        start=(eo == 0),
                        stop=(eo == EO - 1),
                    )
            if 0 < eo < 6:
                # discardable warm-keepers while the next pair is in flight
                for f in range(2):
                    nc.tensor.matmul(
                        fill0[:, :],
                        lhsT=w_encT_sb[:, eo, 0:128],
                        rhs=encT_tiles[0][:, eo, 0:512],
                        start=True,
                        stop=True,
                    )
        for ao in range(2):
            ens = []
            for nt in range(NT):
                energy = en_pool.tile(
                    [P, 512], BF16, tag="energy", name=f"en0_{ao}_{nt}"
                )
                nc.scalar.activation(
                    energy[:], eps01[ao][nt][:], AF.Tanh,
                    bias=bias_sb[:, ao, 0:1],
                )
                ens.append(energy)
            emit_acc(ao, ens, on_dve=True)

        for b in range(B_LOC):
            if b > 0:
                # mask rows for this example into the init tiles' row 0; the
                # previous example's ao=0 accumulation (their only reader)
                # is already behind us in the vector FIFO
                for nt in range(NT):
                    nc.vector.tensor_copy(
                        mbinit[nt][0:1, :],
                        mb_sb[0:1, b * TX + nt * 512 : b * TX + (nt + 1) * 512],
                    )

            sc_ps = [
                vec_psum.tile([1, 512], F32, tag="sc", bufs=2, name=f"sc{b}_{nt}")
                for nt in range(NT)
            ]

            for ao in range(2 if b == 0 else 0, AO):
                eps = [
                    ep_psum.tile([P, 512], F32, tag="ep", name=f"ep{b}_{ao}_{nt}")
                    for nt in range(NT)
                ]
                ens = emit_energy(b, ao, eps)
                if pend_ones is not None and ao == 0:
                    # previous example's score sum: emitted after this pass's
                    # energy matmuls (so the PE never stalls on the tanh->acc
                    # chain) but BEFORE this example's first acc write below
                    pend_ones()
                    pend_ones = None
                emit_acc(ao, ens, last=(b == B_LOC - 1 and ao == AO - 1),
                         on_dve=(b == 0 and ao < 4))
                if b == 0 and ao == 3:
                    dec_proj_pass(1)
                if pend_ctx is not None and ao == 1:
                    pend_ctx()
                    pend_ctx = None
                if pend_ctx_tp is not None and ao == 7:
                    pend_ctx_tp()
                    pend_ctx_tp = None

            def emit_ones(sc_ps=sc_ps):
                # cross-partition score sum: split acc into bf16 hi+lo, then
                # ones^T @ hi + ones^T @ lo accumulate into the score PSUM
                fin = (AO - 1) % 2
                for nt in range(NT):
                    nc.vector.tensor_copy(acc_hi[nt][:], acc_nt[nt][fin][:])
                    nc.vector.tensor_tensor(
                        out=acc_lo[nt][:], in0=acc_nt[nt][fin][:],
                        in1=acc_hi[nt][:], op=ALU.subtract,
                    )
                for nt in range(NT):
                    nc.tensor.matmul(
                        sc_ps[nt][:], lhsT=ones_col[:], rhs=acc_hi[nt][:],
                        start=True, stop=False,
                    )
                    nc.tensor.matmul(
                        sc_ps[nt][:], lhsT=ones_col[:], rhs=acc_lo[nt][:],
                        start=False, stop=True,
                    )

            # masked softmax (see module docstring: no max shift needed)
            exp_bf = rowp.tile([1, TX], BF16, tag="erow", name=f"exp{b}")
            asum = small.tile([1, NT], F32, tag="asum", name=f"asum{b}")
            ssum = small.tile([1, 1], F32, tag="ssum", name=f"ssum{b}")
            rsum = small.tile([1, 1], F32, tag="rsum", name=f"rsum{b}")
            alpha_row = rowp.tile([1, TX], F32, tag="arow", bufs=1, name=f"alpha{b}")
            alpha_bf = rowp.tile([1, TX], BF16, tag="abrow", bufs=1, name=f"alphabf{b}")

            def emit_exp(sc_ps=sc_ps, exp_bf=exp_bf, asum=asum):
                for nt in range(NT):
                    hs = slice(nt * 512, (nt + 1) * 512)
                    nc.scalar.activation(
                        exp_bf[:, hs], sc_ps[nt][:], AF.Exp,
                        accum_out=asum[:, nt : nt + 1],
                    )

            def emit_sm_tail(b=b, exp_bf=exp_bf, asum=asum, ssum=ssum,
                             rsum=rsum, alpha_row=alpha_row, alpha_bf=alpha_bf):
                nc.vector.tensor_add(out=ssum[:], in0=asum[:, 0:1], in1=asum[:, 1:2])
                nc.vector.reciprocal(rsum[:], ssum[:])
                nc.vector.tensor_scalar_mul(alpha_row[:], exp_bf[:], rsum[:])
                nc.sync.dma_start(alpha_out[b : b + 1, :], alpha_row[:])
                nc.vector.tensor_scalar_mul(alpha_bf[:], exp_bf[:], rsum[:])

            def emit_softmax():
                emit_exp()
                emit_sm_tail()

            if b < B_LOC - 1:
                def pend_ones_fn(emit_ones=emit_ones, emit_softmax=emit_softmax):
                    emit_ones()
                    emit_softmax()
                pend_ones = pend_ones_fn

                def emit_ctx(b=b, alpha_bf=alpha_bf, rsum=rsum):
                    nonlocal pend_ctx_tp
                    # context^T[e-part, eo] = sum_t alpha[t] encT[e, t]:
                    # alpha broadcast to 128 partitions by a K=1 matmul pair,
                    # then one fused multiply+reduce per e-chunk on the DVE.
                    encT_sb = encT_tiles[b]
                    alpha_bc = rowp.tile([P, TX], BF16, tag="abc", name=f"abc{b}")
                    for nt in range(NT):
                        hs = slice(nt * 512, (nt + 1) * 512)
                        bc_ps = vec_psum.tile(
                            [P, 512], F32, tag="vec", bufs=2, name=f"bc{b}_{nt}"
                        )
                        nc.tensor.matmul(
                            bc_ps[:], lhsT=ones_row[:], rhs=alpha_bf[:, hs],
                            start=True, stop=True,
                        )
                        nc.vector.tensor_copy(alpha_bc[:, hs], bc_ps[:])
                    prod = big.tile([P, EO, TX], BF16, tag="prodbig", bufs=1, name=f"pr{b}")
                    ctxT = small.tile([P, EO], F32, tag="ctxT", name=f"ctxT{b}")
                    for eo in range(EO):
                        nc.vector.tensor_mul(
                            out=prod[:, eo], in0=encT_sb[:, eo], in1=alpha_bc[:]
                        )
                    nc.vector.tensor_reduce(
                        ctxT[:], prod[:], axis=mybir.AxisListType.X, op=ALU.add
                    )

                    def emit_ctx_tp(b=b, ctxT=ctxT):
                        tp_ps = ep_psum.tile([EO, P], F32, tag="ep", name=f"ctp{b}")
                        nc.tensor.transpose(tp_ps[:], ctxT[:], ident128[:])
                        ctx_row = small.tile([EO, P], F32, tag="ctxrow", name=f"cr{b}")
                        nc.vector.tensor_copy(ctx_row[:], tp_ps[:])
                        nc.sync.dma_start(
                            ctx_out[b].rearrange("(eo p) -> eo p", p=P), ctx_row[:]
                        )

                    pend_ctx_tp = emit_ctx_tp

                pend_ctx = emit_ctx
                continue

            # ---- b=3 serial tail ------------------------------------------
            # PE: scores -> exp -> 8 column-transposes of the exp row -> 8
            # accumulating [1,512] matmuls vs natural-layout enc3 (e 0-511).
            # DVE in parallel: exp broadcast + fused reduce on encT3
            # (e 512-1023). 1/sum folds into both evacuations.
            # discardable warm-keepers BEFORE the ones-matmuls: the
            # vmul->add->cast chain takes ~2us; keep the PE at 8/8
            fill_ps = vec_psum.tile([P, 512], F32, tag="vec", bufs=2, name="fill3")
            for f in range(10):
                nc.tensor.matmul(
                    fill_ps[:B_LOC, :],
                    lhsT=dec_hT_sb[:, f % DO],
                    rhs=w_decT_sb[:, f % DO, 0:512],
                    start=True,
                    stop=True,
                )
            emit_ones()
            emit_exp()
            for f in range(4):
                nc.tensor.matmul(
                    fill_ps[:B_LOC, :],
                    lhsT=dec_hT_sb[:, (8 + f) % DO],
                    rhs=w_decT_sb[:, (8 + f) % DO, 0:512],
                    start=True,
                    stop=True,
                )

            expT_ps = vec_psum.tile([P, TO, 2], BF16, tag="vec", bufs=2, name="expTps")
            expT = small.tile([P, TO], BF16, tag="expT", name="expT3")
            cxs = [
                vec_psum.tile([1, 512], F32, tag="vec", bufs=2, name=f"cx3_{et}")
                for et in range(2)
            ]
            for half in range(2):
                for k in range(half * 4, half * 4 + 4):
                    nc.tensor.transpose(
                        expT_ps[:, k, 0:1], exp_bf[:, k * P : (k + 1) * P], ident1[:]
                    )
                nc.vector.tensor_copy(
                    expT[:, half * 4 : half * 4 + 4],
                    expT_ps[:, half * 4 : half * 4 + 4, 0],
                )
                for to in range(half * 4, half * 4 + 4):
                    for et in range(2):
                        nc.tensor.matmul(
                            cxs[et][:],
                            lhsT=expT[:, to : to + 1],
                            rhs=enc3_sb[:, to, et * 512 : (et + 1) * 512],
                            start=(to == 0),
                            stop=(to == TO - 1),
                        )
            emit_sm_tail()
            ctx_row = small.tile([1, E], F32, tag="ctxrow", name="cr3")
            for et in range(2):
                nc.vector.tensor_scalar_mul(
                    ctx_row[:, et * 512 : (et + 1) * 512], cxs[et][:], rsum[:]
                )
            nc.sync.dma_start(ctx_out[b : b + 1, :], ctx_row[:])

        if pend_ctx_tp is not None:
            pend_ctx_tp()
        if pend_ctx is not None:
            pend_ctx()
        if pend_ctx_tp is not None:
            pend_ctx_tp()

    nc.compile()
    return nc


_NC = None


def _get_nc():
    global _NC
    if _NC is None:
        _NC = build_nc()
    return _NC


def make_in_maps(dec_hidden, enc_outputs, mask, W_w, W_b, v_w):
    import ml_dtypes

    BF = ml_dtypes.bfloat16
    dec_hidden = np.asarray(dec_hidden, np.float32)
    enc_outputs = np.asarray(enc_outputs, np.float32)
    W_w = np.asarray(W_w, np.float32)
    W_b = np.asarray(W_b, np.float32)
    v_w = np.asarray(v_w, np.float32)
    mb = (np.asarray(mask).astype(np.float32) - 1.0) * MASK_PEN

    encT = np.ascontiguousarray(enc_outputs.transpose(0, 2, 1).astype(BF))
    w_encT = np.ascontiguousarray(W_w[:, D:].T.astype(BF))
    w_decT = np.ascontiguousarray(W_w[:, :D].T.astype(BF))
    wb8 = np.ascontiguousarray(W_b.reshape(AO, P).T)
    v8 = np.ascontiguousarray(v_w[0].reshape(AO, P).T.astype(np.float32))

    in_maps = []
    for c in range(N_CORES):
        sl = slice(B_LOC * c, B_LOC * (c + 1))
        in_maps.append(
            {
                "encT": encT[sl],
                "enc3": np.ascontiguousarray(
                    enc_outputs[B_LOC * c + 3].astype(BF)
                ),
                "w_encT": w_encT,
                "w_decT": w_decT,
                "dec_hT": np.ascontiguousarray(dec_hidden[sl].T.astype(BF)),
                "v8": v8,
                "wb8": wb8,
                "mb": np.ascontiguousarray(mb[sl].reshape(1, B_LOC * TX)),
            }
        )
    return in_maps


def kernel(dec_hidden, enc_outputs, mask, W_w, W_b, v_w):
    from concourse.bass_utils import run_bass_kernel_spmd

    assert enc_outputs.shape == (N_CORES * B_LOC, TX, E), enc_outputs.shape
    nc = _get_nc()
    in_maps = make_in_maps(dec_hidden, enc_outputs, mask, W_w, W_b, v_w)
    res = run_bass_kernel_spmd(nc, in_maps, list(range(N_CORES))).results
    context = np.concatenate([res[c]["context"] for c in range(N_CORES)], axis=0)
    alpha = np.concatenate([res[c]["alpha"] for c in range(N_CORES)], axis=0)
    return context, alpha
